# revision 26
# baseline (speedup 1.0000x reference)
"""Multi-head causal self-attention for TRN2, 8 NeuronCores.

Sharding: core i handles (batch b = i//2, head-group g = i%2); each head-group
is 8 of the 16 heads.  Per core everything is computed in "transposed" space so
no on-device transposes are needed.

v2 changes vs the 405us baseline:
  - all big inputs (x^T, W_qkv, W_v, W_proj) are pre-cast to bf16 on the host
    and DMA'd straight into the compute tiles: no on-device fp32->bf16 casts
    (was ~120us of DVE CAST work) and half the input HBM traffic.
  - QKV PSUM->SBUF writes (with bias) moved from ACT to DVE tensor_scalar_add
    so ACT does nothing but the softmax exp.
  - diagonal-chunk strip trimming: for the chunk on the causal diagonal only
    the valid query column range [s*128, 512) is computed by QK / exp / PV,
    and the causal mask multiply shrinks to one 128x128 tril block per head.
  - normalization is deferred (pending-callback pipeline): sums copy (DVE),
    batched reciprocal per head-pair (DVE), reciprocal row broadcast on the
    GPSIMD/Pool engine (partition_broadcast, SBUF only), and a single fused
    po[0:64] * bcast -> yT multiply per head (DVE).  No PSUM bank for the
    broadcast and no o_sb staging copy.
  - PSUM = exactly 8 banks: ss pool 2x[128,1024] (4 banks) + po pool
    4x[128,512] (4 banks).  The output projection runs in short bursts
    between head-pairs borrowing ss-pool tiles.
  - V-staging copies into the [V_h|1] layout are one strided-AP copy per
    tk-chunk instead of 8.
"""

import numpy as np
import ml_dtypes
from contextlib import ExitStack

import concourse.bass as bass
import concourse.mybir as mybir
import concourse.tile as tile
from concourse import bacc
from concourse.bass_utils import run_bass_kernel_spmd

B, T, D, H = 4, 2048, 1024, 16
DK = 64            # head dim
HL = 8             # heads per core
DL = HL * DK       # 512 local head dims per core
N_CORES = 8

F32 = mybir.dt.float32
F32R = mybir.dt.float32r
BF16 = mybir.dt.bfloat16
EXP = mybir.ActivationFunctionType.Exp

TQ = 512           # tq block size
TKC = 128          # tk chunk size
NQB = T // TQ      # 4
NKC = T // TKC     # 16
NDCH = D // 128    # 8 contraction chunks over D
VSW = HL * 65 + 64  # staged-V width: 8*[V_h|1] + ones tail pad for M=128 lhsT

_CACHE = {}


def _build(causal: bool):
    nc = bacc.Bacc("TRN2", target_bir_lowering=False, debug=False,
                   num_devices=N_CORES)
    xT_d = nc.dram_tensor("xT", [D, T], BF16, kind="ExternalInput").ap()
    wqk_d = nc.dram_tensor("wqk", [D, 2 * DL], BF16, kind="ExternalInput").ap()
    wv_d = nc.dram_tensor("wv", [D, DL], BF16, kind="ExternalInput").ap()
    bqk_d = nc.dram_tensor("bqk", [2 * DL // 128, 128, 1], F32,
                           kind="ExternalInput").ap()
    bv_d = nc.dram_tensor("bv", [1, DL], F32, kind="ExternalInput").ap()
    wp_d = nc.dram_tensor("wproj", [DL, D], BF16, kind="ExternalInput").ap()
    tril_d = nc.dram_tensor("tril2", [TKC, 2 * TKC], BF16,
                            kind="ExternalInput").ap()
    out_d = nc.dram_tensor("out", [T, D], F32, kind="ExternalOutput").ap()

    with tile.TileContext(nc) as tc, ExitStack() as top:
        persist = top.enter_context(tc.tile_pool(name="persist", bufs=1))

        qTp = [persist.tile([128, T], BF16, tag=f"qTp{h}", name=f"qTp{h}")
               for h in range(HL)]      # per-head, zero-padded other half
        kT = [persist.tile([128, T], BF16, tag=f"kT{i}", name=f"kT{i}")
              for i in range(4)]        # head-pair packed
        vs = [persist.tile([128, VSW], BF16, tag=f"vs{t}", name=f"vs{t}")
              for t in range(NKC)]
        yT = [persist.tile([128, T], BF16, tag=f"yT{i}", name=f"yT{i}")
              for i in range(4)]
        wp_sb = [persist.tile([128, D], BF16, tag=f"wp{k}", name=f"wp{k}")
                 for k in range(4)]
        ones_f = persist.tile([1, 128], F32, tag="ones_f", name="ones_f")
        ones_rt = persist.tile([1, 128], F32R, tag="ones_rt", name="ones_rt")
        bqk_sb = [persist.tile([128, 1], F32, tag=f"bqk{m}", name=f"bqk{m}")
                  for m in range(8)]
        bv_f = persist.tile([1, DL], F32, tag="bv_f", name="bv_f")
        bv_rt = persist.tile([1, DL], F32R, tag="bv_rt", name="bv_rt")
        tril2 = persist.tile([TKC, 2 * TKC], BF16, tag="tril2", name="tril2")

        # constants + weight DMAs (gpsimd queue), x DMAs (sync queue)
        nc.vector.memset(ones_f[:], 1.0)
        nc.vector.tensor_copy(ones_rt[:], ones_f[:])
        for m in range(8):
            nc.gpsimd.dma_start(bqk_sb[m][:], bqk_d[m])
        nc.gpsimd.dma_start(bv_f[:], bv_d)
        nc.vector.tensor_copy(bv_rt[:], bv_f[:])
        if causal:
            nc.gpsimd.dma_start(tril2[:], tril_d)

        # zero-pad halves of qTp (read by attention QK matmuls) and the
        # [V|1] ones layout of vs -- on Pool, it's idle during phase 1
        for h in range(HL):
            pad = slice(64, 128) if h % 2 == 0 else slice(0, 64)
            nc.gpsimd.memset(qTp[h][pad, :], 0.0)
        for t in range(NKC):
            nc.gpsimd.memset(vs[t][:], 1.0)

        # ---------------- phase 1: QKV projections ----------------
        with ExitStack() as ph1:
            xpool = ph1.enter_context(tc.tile_pool(name="xpool", bufs=1))
            wpool = ph1.enter_context(tc.tile_pool(name="wpool", bufs=1))
            ps1 = ph1.enter_context(tc.tile_pool(name="ps1", bufs=3,
                                                 space="PSUM"))
            psv = ph1.enter_context(tc.tile_pool(name="psv", bufs=2,
                                                 space="PSUM"))

            wqk_sb, wv_sb = [], []
            for d in range(NDCH):
                wr = wpool.tile([128, 2 * DL], BF16, tag=f"wqk{d}",
                                name=f"wqk{d}")
                nc.gpsimd.dma_start(wr[:], wqk_d[d * 128:(d + 1) * 128, :])
                wqk_sb.append(wr)
                wvr = wpool.tile([128, DL], BF16, tag=f"wv{d}", name=f"wv{d}")
                nc.gpsimd.dma_start(wvr[:], wv_d[d * 128:(d + 1) * 128, :])
                wv_sb.append(wvr)
            for k in range(4):
                nc.gpsimd.dma_start(wp_sb[k][:],
                                    wp_d[k * 128:(k + 1) * 128, :])

            # x chunks: per (j, d) so compute starts after the first 8
            xr = [[None] * NDCH for _ in range(NQB)]
            for j in range(NQB):
                for d in range(NDCH):
                    xt = xpool.tile([128, TQ], BF16, tag=f"x{j}_{d}",
                                    name=f"x{j}_{d}")
                    nc.sync.dma_start(
                        xt[:], xT_d[d * 128:(d + 1) * 128,
                                    j * TQ:(j + 1) * TQ])
                    xr[j][d] = xt

            ones_r = ones_rt[:]
            bv_r = bv_rt[:]

            for j in range(NQB):
                jsl = slice(j * TQ, (j + 1) * TQ)
                for m in range(8):
                    ps = ps1.tile([128, TQ], F32, tag="psqk",
                                  name=f"psqk{j}_{m}")
                    for d in range(NDCH):
                        nc.tensor.matmul(
                            ps[:], wqk_sb[d][:, m * 128:(m + 1) * 128],
                            xr[j][d][:], start=(d == 0), stop=(d == NDCH - 1))
                    if m < 4:
                        nc.vector.tensor_scalar_add(
                            qTp[2 * m][0:64, jsl], ps[0:64, :],
                            bqk_sb[m][0:64])
                        nc.vector.tensor_scalar_add(
                            qTp[2 * m + 1][64:128, jsl], ps[64:128, :],
                            bqk_sb[m][64:128])
                    else:
                        nc.vector.tensor_scalar_add(
                            kT[m - 4][:, jsl], ps[:], bqk_sb[m][:])

                for tt in range(4 * j, 4 * j + 4):
                    c = tt % 4
                    ps = psv.tile([128, DL], F32, tag="psv", name=f"psv{tt}")
                    for d in range(NDCH):
                        nc.tensor.matmul(
                            ps[:], xr[j][d][:, c * 128:(c + 1) * 128],
                            wv_sb[d][:], start=(d == 0), stop=False)
                    nc.tensor.matmul(ps[:], ones_r[:, 0:128], bv_r,
                                     start=False, stop=True)
                    for h in range(HL):
                        nc.vector.tensor_copy(
                            vs[tt][:, h * 65:h * 65 + 64],
                            ps[:, h * 64:(h + 1) * 64])

        # -------- phase 2: attention + projection --------
        with ExitStack() as ph2:
            ps_s = ph2.enter_context(tc.tile_pool(name="ps_s", bufs=3,
                                                  space="PSUM"))
            ps_o = ph2.enter_context(tc.tile_pool(name="ps_o", bufs=2,
                                                  space="PSUM"))
            ppool = ph2.enter_context(tc.tile_pool(name="ppool", bufs=6))
            npool = ph2.enter_context(tc.tile_pool(name="npool", bufs=2))
            opool = ph2.enter_context(tc.tile_pool(name="opool", bufs=3))

            pending = []

            def pop_pending():
                if pending:
                    pending.pop(0)()

            def norm_cbs(j, i, poA, poB):
                """Deferred normalization of head pair (2i, 2i+1) of block
                j: divide po rows 0:64 by the softmax sums in row 64.
                recip -> f32r cast -> PE ones-broadcast into a borrowed
                ss-pool tile -> po*pb multiply into yT."""
                jsl = slice(j * TQ, (j + 1) * TQ)
                sumA = npool.tile([1, TQ], F32, tag="sumA", name=f"sa{j}_{i}")
                sumB = npool.tile([1, TQ], F32, tag="sumB", name=f"sb{j}_{i}")
                recAB = npool.tile([1, 2 * TQ], F32, tag="recAB",
                                   name=f"rc{j}_{i}")
                scr = npool.tile([1, TQ], F32, tag="scr", name=f"sc{j}_{i}")
                pbAB = npool.tile([64, 2 * TQ], F32, tag="pbAB",
                                  name=f"pb{j}_{i}")

                osbA = npool.tile([64, TQ], BF16, tag="osbA",
                                  name=f"oa{j}_{i}")
                osbB = npool.tile([64, TQ], BF16, tag="osbB",
                                  name=f"ob{j}_{i}")

                def cb1():     # last reads of poA -> early release
                    nc.vector.tensor_copy(sumA[:], poA[64:65, :])
                    nc.vector.tensor_copy(osbA[:], poA[0:64, :])

                def cb2():     # last reads of poB -> early release
                    nc.vector.tensor_copy(sumB[:], poB[64:65, :])
                    nc.vector.tensor_copy(osbB[:], poB[0:64, :])

                def cb3():
                    nc.vector.reciprocal_approx_accurate(
                        out=recAB[:, 0:TQ], in_=sumA[:], scratch=scr[:])
                    nc.vector.reciprocal_approx_accurate(
                        out=recAB[:, TQ:2 * TQ], in_=sumB[:], scratch=scr[:])

                def cb4():
                    nc.gpsimd.partition_broadcast(pbAB[:], recAB[:])

                def cb5():
                    nc.vector.tensor_mul(yT[i][0:64, jsl], osbA[:],
                                         pbAB[:, 0:TQ])
                    nc.vector.tensor_mul(yT[i][64:128, jsl], osbB[:],
                                         pbAB[:, TQ:2 * TQ])

                return [cb1, cb2, cb3, cb4, cb5]

            def proj_burst(jb):
                """Output projection of block jb: 8 steps, each borrows an
                ss-pool tile for its PSUM accumulation."""
                for t in range(4 * jb, 4 * jb + 4):
                    for nb in range(2):
                        nsl = slice(nb * 512, (nb + 1) * 512)
                        ps = ps_s.tile([TKC, 2 * TQ], F32, tag="ss",
                                       name=f"ssp{t}_{nb}")
                        for k in range(4):
                            nc.tensor.matmul(
                                ps[:, 0:TQ],
                                yT[k][:, t * 128:(t + 1) * 128],
                                wp_sb[k][:, nsl], start=(k == 0),
                                stop=(k == 3))
                        ot = opool.tile([128, TQ], F32, tag="ot",
                                        name=f"ot{t}_{nb}")
                        nc.vector.tensor_copy(ot[:], ps[:, 0:TQ])
                        nc.sync.dma_start(out_d[t * 128:(t + 1) * 128, nsl],
                                          ot[:])
                        pop_pending()

            for j in range(NQB):
                jsl = slice(j * TQ, (j + 1) * TQ)
                cs = list(range(4 * (j + 1))) if causal else list(range(NKC))
                for i in range(4):          # head pair (2i, 2i+1)
                    if i == 2 and j > 0:
                        proj_burst(j - 1)
                    hA, hB = 2 * i, 2 * i + 1
                    poA = ps_o.tile([128, TQ], F32, tag="po",
                                    name=f"poA{j}_{i}")
                    poB = ps_o.tile([128, TQ], F32, tag="po",
                                    name=f"poB{j}_{i}")

                    pendq = []    # pipeline: PV(c) emitted after QK(c+2)
                    first_pv = [True]

                    def emit_pv(pc, ppt, pq0, stop):
                        st = first_pv[0]
                        first_pv[0] = False
                        nc.tensor.matmul(
                            poA[:, pq0:TQ],
                            vs[pc][:, hA * 65:hA * 65 + 128],
                            ppt[:, pq0:TQ], start=st, stop=stop,
                            skip_group_check=True)
                        nc.tensor.matmul(
                            poB[:, pq0:TQ],
                            vs[pc][:, hB * 65:hB * 65 + 128],
                            ppt[:, TQ + pq0:2 * TQ], start=st, stop=stop,
                            skip_group_check=True)

                    for ci, c in enumerate(cs):
                        diag = causal and c >= 4 * j
                        q0 = (c - 4 * j) * TKC if diag else 0
                        csl = slice(c * TKC, (c + 1) * TKC)
                        ss = ps_s.tile([TKC, 2 * TQ], F32, tag="ss",
                                       name=f"ss{j}_{i}_{c}")
                        nc.tensor.matmul(
                            ss[:, q0:TQ], kT[i][:, csl],
                            qTp[hA][:, j * TQ + q0:(j + 1) * TQ],
                            start=True, stop=True)
                        nc.tensor.matmul(
                            ss[:, TQ + q0:2 * TQ], kT[i][:, csl],
                            qTp[hB][:, j * TQ + q0:(j + 1) * TQ],
                            start=True, stop=True)
                        pt = ppool.tile([TKC, 2 * TQ], BF16, tag="pt",
                                        name=f"pt{j}_{i}_{c}")
                        if q0 == 0:
                            nc.scalar.activation(pt[:], ss[:], EXP,
                                                 scale=0.125)
                        else:
                            nc.scalar.activation(pt[:, q0:TQ], ss[:, q0:TQ],
                                                 EXP, scale=0.125)
                            nc.scalar.activation(pt[:, TQ + q0:2 * TQ],
                                                 ss[:, TQ + q0:2 * TQ],
                                                 EXP, scale=0.125)
                        if diag:
                            # tril mask on the one boundary 128-col block
                            nc.vector.tensor_mul(pt[:, q0:q0 + TKC],
                                                 pt[:, q0:q0 + TKC],
                                                 tril2[:, 0:TKC])
                            nc.vector.tensor_mul(
                                pt[:, TQ + q0:TQ + q0 + TKC],
                                pt[:, TQ + q0:TQ + q0 + TKC],
                                tril2[:, TKC:2 * TKC])
                        if len(pendq) == 3:
                            emit_pv(*pendq.pop(0), stop=False)
                        pop_pending()
                        pendq.append((c, pt, q0))
                    while pendq:
                        emit_pv(*pendq.pop(0), stop=(len(pendq) == 0))

                    cbs = norm_cbs(j, i, poA, poB)
                    if j == NQB - 1 and i == 3:
                        for cb in cbs:   # last pair: start the chain now
                            cb()
                    else:
                        pending.extend(cbs)

            for fn in list(pending):   # flush last block's normalization
                pop_pending()
            proj_burst(NQB - 1)

    nc.compile()
    return nc


def _get_nc(causal: bool):
    if causal not in _CACHE:
        _CACHE[causal] = _build(causal)
    return _CACHE[causal]


def _host_tril2() -> np.ndarray:
    i = np.arange(TKC)[:, None]
    jj = np.arange(TKC)[None, :]
    blk = (jj >= i).astype(np.float32)
    return np.ascontiguousarray(
        np.concatenate([blk, blk], axis=1).astype(ml_dtypes.bfloat16))


def _make_in_maps(x, W_qkv, b_qkv, W_proj):
    tril_np = _host_tril2()
    bf = ml_dtypes.bfloat16
    in_maps = []
    for core in range(N_CORES):
        b, g = core // 2, core % 2
        qc = slice(g * DL, (g + 1) * DL)
        kc = slice(D + g * DL, D + (g + 1) * DL)
        vc = slice(2 * D + g * DL, 2 * D + (g + 1) * DL)
        in_maps.append({
            "xT": np.ascontiguousarray(x[b].T.astype(bf)),
            "wqk": np.ascontiguousarray(
                np.concatenate([W_qkv[:, qc], W_qkv[:, kc]],
                               axis=1).astype(bf)),
            "wv": np.ascontiguousarray(W_qkv[:, vc].astype(bf)),
            "bqk": np.ascontiguousarray(
                np.concatenate([b_qkv[qc], b_qkv[kc]]).reshape(8, 128, 1)),
            "bv": np.ascontiguousarray(b_qkv[vc].reshape(1, DL)),
            "wproj": np.ascontiguousarray(
                W_proj[g * DL:(g + 1) * DL, :].astype(bf)),
            "tril2": tril_np,
        })
    return in_maps


def kernel(x, mask, W_qkv, b_qkv, W_proj, b_proj):
    x = np.asarray(x, dtype=np.float32)
    mask2d = np.asarray(mask, dtype=np.int32).reshape(T, T)
    W_qkv = np.asarray(W_qkv, dtype=np.float32)
    b_qkv = np.asarray(b_qkv, dtype=np.float32)
    W_proj = np.asarray(W_proj, dtype=np.float32)
    b_proj = np.asarray(b_proj, dtype=np.float32)

    if np.array_equal(mask2d, np.tril(np.ones((T, T), dtype=np.int32))):
        causal = True
    elif np.all(mask2d == 1):
        causal = False
    else:
        raise NotImplementedError("only causal (tril) or all-ones masks")

    nc = _get_nc(causal)
    in_maps = _make_in_maps(x, W_qkv, b_qkv, W_proj)
    res = run_bass_kernel_spmd(nc, in_maps, core_ids=list(range(N_CORES)))
    out = np.empty((B, T, D), dtype=np.float32)
    for b in range(B):
        out[b] = (res.results[2 * b]["out"] + res.results[2 * b + 1]["out"]
                  + b_proj[None, :])
    return out


# revision 29
# speedup vs baseline: 1.0007x; 1.0007x over previous
"""Multi-head causal self-attention for TRN2, 8 NeuronCores.

Sharding: core i handles (batch b = i//2, head-group g = i%2); each head-group
is 8 of the 16 heads.  Per core everything is computed in "transposed" space so
no on-device transposes are needed.

v2 changes vs the 405us baseline:
  - all big inputs (x^T, W_qkv, W_v, W_proj) are pre-cast to bf16 on the host
    and DMA'd straight into the compute tiles: no on-device fp32->bf16 casts
    (was ~120us of DVE CAST work) and half the input HBM traffic.
  - QKV PSUM->SBUF writes (with bias) moved from ACT to DVE tensor_scalar_add
    so ACT does nothing but the softmax exp.
  - diagonal-chunk strip trimming: for the chunk on the causal diagonal only
    the valid query column range [s*128, 512) is computed by QK / exp / PV,
    and the causal mask multiply shrinks to one 128x128 tril block per head.
  - normalization is deferred (pending-callback pipeline): sums copy (DVE),
    batched reciprocal per head-pair (DVE), reciprocal row broadcast on the
    GPSIMD/Pool engine (partition_broadcast, SBUF only), and a single fused
    po[0:64] * bcast -> yT multiply per head (DVE).  No PSUM bank for the
    broadcast and no o_sb staging copy.
  - PSUM = exactly 8 banks: ss pool 2x[128,1024] (4 banks) + po pool
    4x[128,512] (4 banks).  The output projection runs in short bursts
    between head-pairs borrowing ss-pool tiles.
  - V-staging copies into the [V_h|1] layout are one strided-AP copy per
    tk-chunk instead of 8.
"""

import numpy as np
import ml_dtypes
from contextlib import ExitStack

import concourse.bass as bass
import concourse.mybir as mybir
import concourse.tile as tile
from concourse import bacc
from concourse.bass_utils import run_bass_kernel_spmd

B, T, D, H = 4, 2048, 1024, 16
DK = 64            # head dim
HL = 8             # heads per core
DL = HL * DK       # 512 local head dims per core
N_CORES = 8

F32 = mybir.dt.float32
F32R = mybir.dt.float32r
BF16 = mybir.dt.bfloat16
EXP = mybir.ActivationFunctionType.Exp

TQ = 512           # tq block size
TKC = 128          # tk chunk size
NQB = T // TQ      # 4
NKC = T // TKC     # 16
NDCH = D // 128    # 8 contraction chunks over D
VSW = HL * 65 + 64  # staged-V width: 8*[V_h|1] + ones tail pad for M=128 lhsT

_CACHE = {}


def _build(causal: bool):
    nc = bacc.Bacc("TRN2", target_bir_lowering=False, debug=False,
                   num_devices=N_CORES)
    xT_d = nc.dram_tensor("xT", [D, T], BF16, kind="ExternalInput").ap()
    wqk_d = nc.dram_tensor("wqk", [D, 2 * DL], BF16, kind="ExternalInput").ap()
    wv_d = nc.dram_tensor("wv", [D, DL], BF16, kind="ExternalInput").ap()
    bqk_d = nc.dram_tensor("bqk", [2 * DL // 128, 128, 1], F32,
                           kind="ExternalInput").ap()
    bv_d = nc.dram_tensor("bv", [1, DL], F32, kind="ExternalInput").ap()
    wp_d = nc.dram_tensor("wproj", [DL, D], BF16, kind="ExternalInput").ap()
    tril_d = nc.dram_tensor("tril2", [TKC, 2 * TKC], BF16,
                            kind="ExternalInput").ap()
    out_d = nc.dram_tensor("out", [T, D], F32, kind="ExternalOutput").ap()

    with tile.TileContext(nc) as tc, ExitStack() as top:
        persist = top.enter_context(tc.tile_pool(name="persist", bufs=1))

        qTp = [persist.tile([128, T], BF16, tag=f"qTp{h}", name=f"qTp{h}")
               for h in range(HL)]      # per-head, zero-padded other half
        kT = [persist.tile([128, T], BF16, tag=f"kT{i}", name=f"kT{i}")
              for i in range(4)]        # head-pair packed
        vs = [persist.tile([128, VSW], BF16, tag=f"vs{t}", name=f"vs{t}")
              for t in range(NKC)]
        yT = [persist.tile([128, T], BF16, tag=f"yT{i}", name=f"yT{i}")
              for i in range(4)]
        wp_sb = [persist.tile([128, D], BF16, tag=f"wp{k}", name=f"wp{k}")
                 for k in range(4)]
        ones_f = persist.tile([1, 128], F32, tag="ones_f", name="ones_f")
        ones_rt = persist.tile([1, 128], F32R, tag="ones_rt", name="ones_rt")
        bqk_sb = [persist.tile([128, 1], F32, tag=f"bqk{m}", name=f"bqk{m}")
                  for m in range(8)]
        bv_f = persist.tile([1, DL], F32, tag="bv_f", name="bv_f")
        bv_rt = persist.tile([1, DL], F32R, tag="bv_rt", name="bv_rt")
        tril2 = persist.tile([TKC, 2 * TKC], BF16, tag="tril2", name="tril2")

        # constants + weight DMAs (gpsimd queue), x DMAs (sync queue)
        nc.vector.memset(ones_f[:], 1.0)
        nc.vector.tensor_copy(ones_rt[:], ones_f[:])
        for m in range(8):
            nc.gpsimd.dma_start(bqk_sb[m][:], bqk_d[m])
        nc.gpsimd.dma_start(bv_f[:], bv_d)
        nc.vector.tensor_copy(bv_rt[:], bv_f[:])
        if causal:
            nc.gpsimd.dma_start(tril2[:], tril_d)

        # zero-pad halves of qTp (read by attention QK matmuls) and the
        # [V|1] ones layout of vs -- on Pool, it's idle during phase 1
        for h in range(HL):
            pad = slice(64, 128) if h % 2 == 0 else slice(0, 64)
            nc.gpsimd.memset(qTp[h][pad, :], 0.0)
        for t in range(NKC):
            nc.gpsimd.memset(vs[t][:], 1.0)

        # ---------------- phase 1: QKV projections ----------------
        with ExitStack() as ph1:
            xpool = ph1.enter_context(tc.tile_pool(name="xpool", bufs=1))
            wpool = ph1.enter_context(tc.tile_pool(name="wpool", bufs=1))
            ps1 = ph1.enter_context(tc.tile_pool(name="ps1", bufs=3,
                                                 space="PSUM"))
            psv = ph1.enter_context(tc.tile_pool(name="psv", bufs=2,
                                                 space="PSUM"))

            wqk_sb, wv_sb = [], []
            for d in range(NDCH):
                wr = wpool.tile([128, 2 * DL], BF16, tag=f"wqk{d}",
                                name=f"wqk{d}")
                nc.gpsimd.dma_start(wr[:], wqk_d[d * 128:(d + 1) * 128, :])
                wqk_sb.append(wr)
                wvr = wpool.tile([128, DL], BF16, tag=f"wv{d}", name=f"wv{d}")
                nc.gpsimd.dma_start(wvr[:], wv_d[d * 128:(d + 1) * 128, :])
                wv_sb.append(wvr)
            for k in range(4):
                nc.gpsimd.dma_start(wp_sb[k][:],
                                    wp_d[k * 128:(k + 1) * 128, :])

            # x chunks: per (j, d) so compute starts after the first 8
            xr = [[None] * NDCH for _ in range(NQB)]
            for j in range(NQB):
                for d in range(NDCH):
                    xt = xpool.tile([128, TQ], BF16, tag=f"x{j}_{d}",
                                    name=f"x{j}_{d}")
                    nc.sync.dma_start(
                        xt[:], xT_d[d * 128:(d + 1) * 128,
                                    j * TQ:(j + 1) * TQ])
                    xr[j][d] = xt

            ones_r = ones_rt[:]
            bv_r = bv_rt[:]

            for j in range(NQB):
                jsl = slice(j * TQ, (j + 1) * TQ)
                for m in range(8):
                    ps = ps1.tile([128, TQ], F32, tag="psqk",
                                  name=f"psqk{j}_{m}")
                    for d in range(NDCH):
                        nc.tensor.matmul(
                            ps[:], wqk_sb[d][:, m * 128:(m + 1) * 128],
                            xr[j][d][:], start=(d == 0), stop=(d == NDCH - 1))
                    if m < 4:
                        nc.vector.tensor_scalar_add(
                            qTp[2 * m][0:64, jsl], ps[0:64, :],
                            bqk_sb[m][0:64])
                        nc.vector.tensor_scalar_add(
                            qTp[2 * m + 1][64:128, jsl], ps[64:128, :],
                            bqk_sb[m][64:128])
                    else:
                        nc.vector.tensor_scalar_add(
                            kT[m - 4][:, jsl], ps[:], bqk_sb[m][:])

                for tt in range(4 * j, 4 * j + 4):
                    c = tt % 4
                    ps = psv.tile([128, DL], F32, tag="psv", name=f"psv{tt}")
                    for d in range(NDCH):
                        nc.tensor.matmul(
                            ps[:], xr[j][d][:, c * 128:(c + 1) * 128],
                            wv_sb[d][:], start=(d == 0), stop=False)
                    nc.tensor.matmul(ps[:], ones_r[:, 0:128], bv_r,
                                     start=False, stop=True)
                    for h in range(HL):
                        nc.vector.tensor_copy(
                            vs[tt][:, h * 65:h * 65 + 64],
                            ps[:, h * 64:(h + 1) * 64])

        # -------- phase 2: attention + projection --------
        with ExitStack() as ph2:
            ps_s = ph2.enter_context(tc.tile_pool(name="ps_s", bufs=2,
                                                  space="PSUM"))
            ps_o = ph2.enter_context(tc.tile_pool(name="ps_o", bufs=4,
                                                  space="PSUM"))
            ppool = ph2.enter_context(tc.tile_pool(name="ppool", bufs=6))
            npool = ph2.enter_context(tc.tile_pool(name="npool", bufs=2))
            opool = ph2.enter_context(tc.tile_pool(name="opool", bufs=3))

            pending = []

            def pop_pending():
                if pending:
                    pending.pop(0)()

            def norm_cbs(j, i, poA, poB):
                """Deferred normalization of head pair (2i, 2i+1) of block
                j: divide po rows 0:64 by the softmax sums in row 64.
                recip -> f32r cast -> PE ones-broadcast into a borrowed
                ss-pool tile -> po*pb multiply into yT."""
                jsl = slice(j * TQ, (j + 1) * TQ)
                sumA = npool.tile([1, TQ], F32, tag="sumA", name=f"sa{j}_{i}")
                sumB = npool.tile([1, TQ], F32, tag="sumB", name=f"sb{j}_{i}")
                recAB = npool.tile([1, 2 * TQ], F32, tag="recAB",
                                   name=f"rc{j}_{i}")
                scr = npool.tile([1, TQ], F32, tag="scr", name=f"sc{j}_{i}")
                pbAB = npool.tile([64, 2 * TQ], F32, tag="pbAB",
                                  name=f"pb{j}_{i}")

                def cb1():
                    nc.vector.tensor_copy(sumA[:], poA[64:65, :])
                    nc.vector.tensor_copy(sumB[:], poB[64:65, :])

                def cb2():
                    nc.vector.reciprocal_approx_accurate(
                        out=recAB[:, 0:TQ], in_=sumA[:], scratch=scr[:])
                    nc.vector.reciprocal_approx_accurate(
                        out=recAB[:, TQ:2 * TQ], in_=sumB[:], scratch=scr[:])

                def cb3():
                    nc.gpsimd.partition_broadcast(pbAB[:], recAB[:])

                def cb4():
                    nc.vector.tensor_mul(yT[i][0:64, jsl], poA[0:64, :],
                                         pbAB[:, 0:TQ])
                    nc.vector.tensor_mul(yT[i][64:128, jsl], poB[0:64, :],
                                         pbAB[:, TQ:2 * TQ])

                return [cb1, cb2, cb3, cb4]

            def proj_burst(jb):
                """Output projection of block jb: 8 steps, each borrows an
                ss-pool tile for its PSUM accumulation."""
                for t in range(4 * jb, 4 * jb + 4):
                    for nb in range(2):
                        nsl = slice(nb * 512, (nb + 1) * 512)
                        ps = ps_s.tile([TKC, 2 * TQ], F32, tag="ss",
                                       name=f"ssp{t}_{nb}")
                        for k in range(4):
                            nc.tensor.matmul(
                                ps[:, 0:TQ],
                                yT[k][:, t * 128:(t + 1) * 128],
                                wp_sb[k][:, nsl], start=(k == 0),
                                stop=(k == 3))
                        ot = opool.tile([128, TQ], F32, tag="ot",
                                        name=f"ot{t}_{nb}")
                        nc.vector.tensor_copy(ot[:], ps[:, 0:TQ])
                        nc.sync.dma_start(out_d[t * 128:(t + 1) * 128, nsl],
                                          ot[:])
                        pop_pending()

            for j in range(NQB):
                jsl = slice(j * TQ, (j + 1) * TQ)
                cs = list(range(4 * (j + 1))) if causal else list(range(NKC))
                for i in range(4):          # head pair (2i, 2i+1)
                    if i == 2 and j > 0:
                        proj_burst(j - 1)
                    hA, hB = 2 * i, 2 * i + 1
                    poA = ps_o.tile([128, TQ], F32, tag="po",
                                    name=f"poA{j}_{i}")
                    poB = ps_o.tile([128, TQ], F32, tag="po",
                                    name=f"poB{j}_{i}")

                    pendq = []    # pipeline: PV(c) emitted after QK(c+2)
                    first_pv = [True]

                    def emit_pv(pc, ppt, pq0, stop):
                        st = first_pv[0]
                        first_pv[0] = False
                        nc.tensor.matmul(
                            poA[:, pq0:TQ],
                            vs[pc][:, hA * 65:hA * 65 + 128],
                            ppt[:, pq0:TQ], start=st, stop=stop,
                            skip_group_check=True)
                        nc.tensor.matmul(
                            poB[:, pq0:TQ],
                            vs[pc][:, hB * 65:hB * 65 + 128],
                            ppt[:, TQ + pq0:2 * TQ], start=st, stop=stop,
                            skip_group_check=True)

                    for ci, c in enumerate(cs):
                        diag = causal and c >= 4 * j
                        q0 = (c - 4 * j) * TKC if diag else 0
                        csl = slice(c * TKC, (c + 1) * TKC)
                        ss = ps_s.tile([TKC, 2 * TQ], F32, tag="ss",
                                       name=f"ss{j}_{i}_{c}")
                        nc.tensor.matmul(
                            ss[:, q0:TQ], kT[i][:, csl],
                            qTp[hA][:, j * TQ + q0:(j + 1) * TQ],
                            start=True, stop=True)
                        nc.tensor.matmul(
                            ss[:, TQ + q0:2 * TQ], kT[i][:, csl],
                            qTp[hB][:, j * TQ + q0:(j + 1) * TQ],
                            start=True, stop=True)
                        pt = ppool.tile([TKC, 2 * TQ], BF16, tag="pt",
                                        name=f"pt{j}_{i}_{c}")
                        if q0 == 0:
                            nc.scalar.activation(pt[:], ss[:], EXP,
                                                 scale=0.125)
                        else:
                            nc.scalar.activation(pt[:, q0:TQ], ss[:, q0:TQ],
                                                 EXP, scale=0.125)
                            nc.scalar.activation(pt[:, TQ + q0:2 * TQ],
                                                 ss[:, TQ + q0:2 * TQ],
                                                 EXP, scale=0.125)
                        if diag:
                            # tril mask on the one boundary 128-col block
                            nc.vector.tensor_mul(pt[:, q0:q0 + TKC],
                                                 pt[:, q0:q0 + TKC],
                                                 tril2[:, 0:TKC])
                            nc.vector.tensor_mul(
                                pt[:, TQ + q0:TQ + q0 + TKC],
                                pt[:, TQ + q0:TQ + q0 + TKC],
                                tril2[:, TKC:2 * TKC])
                        if len(pendq) == 4:
                            emit_pv(*pendq.pop(0), stop=False)
                        pop_pending()
                        pendq.append((c, pt, q0))
                    while pendq:
                        emit_pv(*pendq.pop(0), stop=(len(pendq) == 0))

                    cbs = norm_cbs(j, i, poA, poB)
                    if j == NQB - 1 and i == 3:
                        for cb in cbs:   # last pair: start the chain now
                            cb()
                    else:
                        pending.extend(cbs)

            for fn in list(pending):   # flush last block's normalization
                pop_pending()
            proj_burst(NQB - 1)

    nc.compile()
    return nc


def _get_nc(causal: bool):
    if causal not in _CACHE:
        _CACHE[causal] = _build(causal)
    return _CACHE[causal]


def _host_tril2() -> np.ndarray:
    i = np.arange(TKC)[:, None]
    jj = np.arange(TKC)[None, :]
    blk = (jj >= i).astype(np.float32)
    return np.ascontiguousarray(
        np.concatenate([blk, blk], axis=1).astype(ml_dtypes.bfloat16))


def _make_in_maps(x, W_qkv, b_qkv, W_proj):
    tril_np = _host_tril2()
    bf = ml_dtypes.bfloat16
    in_maps = []
    for core in range(N_CORES):
        b, g = core // 2, core % 2
        qc = slice(g * DL, (g + 1) * DL)
        kc = slice(D + g * DL, D + (g + 1) * DL)
        vc = slice(2 * D + g * DL, 2 * D + (g + 1) * DL)
        in_maps.append({
            "xT": np.ascontiguousarray(x[b].T.astype(bf)),
            "wqk": np.ascontiguousarray(
                np.concatenate([W_qkv[:, qc], W_qkv[:, kc]],
                               axis=1).astype(bf)),
            "wv": np.ascontiguousarray(W_qkv[:, vc].astype(bf)),
            "bqk": np.ascontiguousarray(
                np.concatenate([b_qkv[qc], b_qkv[kc]]).reshape(8, 128, 1)),
            "bv": np.ascontiguousarray(b_qkv[vc].reshape(1, DL)),
            "wproj": np.ascontiguousarray(
                W_proj[g * DL:(g + 1) * DL, :].astype(bf)),
            "tril2": tril_np,
        })
    return in_maps


def kernel(x, mask, W_qkv, b_qkv, W_proj, b_proj):
    x = np.asarray(x, dtype=np.float32)
    mask2d = np.asarray(mask, dtype=np.int32).reshape(T, T)
    W_qkv = np.asarray(W_qkv, dtype=np.float32)
    b_qkv = np.asarray(b_qkv, dtype=np.float32)
    W_proj = np.asarray(W_proj, dtype=np.float32)
    b_proj = np.asarray(b_proj, dtype=np.float32)

    if np.array_equal(mask2d, np.tril(np.ones((T, T), dtype=np.int32))):
        causal = True
    elif np.all(mask2d == 1):
        causal = False
    else:
        raise NotImplementedError("only causal (tril) or all-ones masks")

    nc = _get_nc(causal)
    in_maps = _make_in_maps(x, W_qkv, b_qkv, W_proj)
    res = run_bass_kernel_spmd(nc, in_maps, core_ids=list(range(N_CORES)))
    out = np.empty((B, T, D), dtype=np.float32)
    for b in range(B):
        out[b] = (res.results[2 * b]["out"] + res.results[2 * b + 1]["out"]
                  + b_proj[None, :])
    return out


# revision 31
# speedup vs baseline: 1.0076x; 1.0069x over previous
"""Multi-head causal self-attention for TRN2, 8 NeuronCores.

Sharding: core i handles (batch b = i//2, head-group g = i%2); each head-group
is 8 of the 16 heads.  Per core everything is computed in "transposed" space so
no on-device transposes are needed.

v2 changes vs the 405us baseline:
  - all big inputs (x^T, W_qkv, W_v, W_proj) are pre-cast to bf16 on the host
    and DMA'd straight into the compute tiles: no on-device fp32->bf16 casts
    (was ~120us of DVE CAST work) and half the input HBM traffic.
  - QKV PSUM->SBUF writes (with bias) moved from ACT to DVE tensor_scalar_add
    so ACT does nothing but the softmax exp.
  - diagonal-chunk strip trimming: for the chunk on the causal diagonal only
    the valid query column range [s*128, 512) is computed by QK / exp / PV,
    and the causal mask multiply shrinks to one 128x128 tril block per head.
  - normalization is deferred (pending-callback pipeline): sums copy (DVE),
    batched reciprocal per head-pair (DVE), reciprocal row broadcast on the
    GPSIMD/Pool engine (partition_broadcast, SBUF only), and a single fused
    po[0:64] * bcast -> yT multiply per head (DVE).  No PSUM bank for the
    broadcast and no o_sb staging copy.
  - PSUM = exactly 8 banks: ss pool 2x[128,1024] (4 banks) + po pool
    4x[128,512] (4 banks).  The output projection runs in short bursts
    between head-pairs borrowing ss-pool tiles.
  - V-staging copies into the [V_h|1] layout are one strided-AP copy per
    tk-chunk instead of 8.
"""

import numpy as np
import ml_dtypes
from contextlib import ExitStack

import concourse.bass as bass
import concourse.mybir as mybir
import concourse.tile as tile
from concourse import bacc
from concourse.bass_utils import run_bass_kernel_spmd

B, T, D, H = 4, 2048, 1024, 16
DK = 64            # head dim
HL = 8             # heads per core
DL = HL * DK       # 512 local head dims per core
N_CORES = 8

F32 = mybir.dt.float32
F32R = mybir.dt.float32r
BF16 = mybir.dt.bfloat16
EXP = mybir.ActivationFunctionType.Exp

TQ = 512           # tq block size
TKC = 128          # tk chunk size
NQB = T // TQ      # 4
NKC = T // TKC     # 16
NDCH = D // 128    # 8 contraction chunks over D
VSW = HL * 65 + 64  # staged-V width: 8*[V_h|1] + ones tail pad for M=128 lhsT

_CACHE = {}


def _build(causal: bool):
    nc = bacc.Bacc("TRN2", target_bir_lowering=False, debug=False,
                   num_devices=N_CORES)
    xT_d = nc.dram_tensor("xT", [D, T], BF16, kind="ExternalInput").ap()
    wqk_d = nc.dram_tensor("wqk", [D, 2 * DL], BF16, kind="ExternalInput").ap()
    wv_d = nc.dram_tensor("wv", [D, DL], BF16, kind="ExternalInput").ap()
    bqk_d = nc.dram_tensor("bqk", [2 * DL // 128, 128, 1], F32,
                           kind="ExternalInput").ap()
    bv_d = nc.dram_tensor("bv", [1, DL], F32, kind="ExternalInput").ap()
    wp_d = nc.dram_tensor("wproj", [DL, D], BF16, kind="ExternalInput").ap()
    tril_d = nc.dram_tensor("tril2", [TKC, 2 * TKC], BF16,
                            kind="ExternalInput").ap()
    out_d = nc.dram_tensor("out", [T, D], F32, kind="ExternalOutput").ap()

    with tile.TileContext(nc) as tc, ExitStack() as top:
        persist = top.enter_context(tc.tile_pool(name="persist", bufs=1))

        qTp = [persist.tile([128, T], BF16, tag=f"qTp{h}", name=f"qTp{h}")
               for h in range(HL)]      # per-head, zero-padded other half
        kT = [persist.tile([128, T], BF16, tag=f"kT{i}", name=f"kT{i}")
              for i in range(4)]        # head-pair packed
        vs = [persist.tile([128, VSW], BF16, tag=f"vs{t}", name=f"vs{t}")
              for t in range(NKC)]
        yT = [persist.tile([128, T], BF16, tag=f"yT{i}", name=f"yT{i}")
              for i in range(4)]
        wp_sb = [persist.tile([128, D], BF16, tag=f"wp{k}", name=f"wp{k}")
                 for k in range(4)]
        ones_f = persist.tile([1, 128], F32, tag="ones_f", name="ones_f")
        ones_rt = persist.tile([1, 128], F32R, tag="ones_rt", name="ones_rt")
        bqk_sb = [persist.tile([128, 1], F32, tag=f"bqk{m}", name=f"bqk{m}")
                  for m in range(8)]
        bv_f = persist.tile([1, DL], F32, tag="bv_f", name="bv_f")
        bv_rt = persist.tile([1, DL], F32R, tag="bv_rt", name="bv_rt")
        tril2 = persist.tile([TKC, 2 * TKC], BF16, tag="tril2", name="tril2")

        # constants + weight DMAs (gpsimd queue), x DMAs (sync queue)
        nc.vector.memset(ones_f[:], 1.0)
        nc.vector.tensor_copy(ones_rt[:], ones_f[:])
        # dummy exp: pulls the ACT_TABLE_LOAD (~1.3us) into phase 1 where
        # the Activation engine is idle, off the phase-transition gap
        warm = persist.tile([1, 8], F32, tag="warm", name="warm")
        nc.scalar.activation(warm[:], ones_f[:, 0:8], EXP, scale=1.0)
        for m in range(8):
            nc.gpsimd.dma_start(bqk_sb[m][:], bqk_d[m])
        nc.gpsimd.dma_start(bv_f[:], bv_d)
        nc.vector.tensor_copy(bv_rt[:], bv_f[:])
        if causal:
            nc.gpsimd.dma_start(tril2[:], tril_d)

        # zero-pad halves of qTp (read by attention QK matmuls) and the
        # [V|1] ones layout of vs -- on Pool, it's idle during phase 1
        for h in range(HL):
            pad = slice(64, 128) if h % 2 == 0 else slice(0, 64)
            nc.gpsimd.memset(qTp[h][pad, :], 0.0)
        for t in range(NKC):
            nc.gpsimd.memset(vs[t][:], 1.0)

        # ---------------- phase 1: QKV projections ----------------
        with ExitStack() as ph1:
            xpool = ph1.enter_context(tc.tile_pool(name="xpool", bufs=1))
            wpool = ph1.enter_context(tc.tile_pool(name="wpool", bufs=1))
            ps1 = ph1.enter_context(tc.tile_pool(name="ps1", bufs=3,
                                                 space="PSUM"))
            psv = ph1.enter_context(tc.tile_pool(name="psv", bufs=2,
                                                 space="PSUM"))

            wqk_sb, wv_sb = [], []
            for d in range(NDCH):
                wr = wpool.tile([128, 2 * DL], BF16, tag=f"wqk{d}",
                                name=f"wqk{d}")
                nc.gpsimd.dma_start(wr[:], wqk_d[d * 128:(d + 1) * 128, :])
                wqk_sb.append(wr)
                wvr = wpool.tile([128, DL], BF16, tag=f"wv{d}", name=f"wv{d}")
                nc.gpsimd.dma_start(wvr[:], wv_d[d * 128:(d + 1) * 128, :])
                wv_sb.append(wvr)
            for k in range(4):
                nc.gpsimd.dma_start(wp_sb[k][:],
                                    wp_d[k * 128:(k + 1) * 128, :])

            # x chunks: per (j, d) so compute starts after the first 8
            xr = [[None] * NDCH for _ in range(NQB)]
            for j in range(NQB):
                for d in range(NDCH):
                    xt = xpool.tile([128, TQ], BF16, tag=f"x{j}_{d}",
                                    name=f"x{j}_{d}")
                    nc.sync.dma_start(
                        xt[:], xT_d[d * 128:(d + 1) * 128,
                                    j * TQ:(j + 1) * TQ])
                    xr[j][d] = xt

            ones_r = ones_rt[:]
            bv_r = bv_rt[:]

            for j in range(NQB):
                jsl = slice(j * TQ, (j + 1) * TQ)
                for m in range(8):
                    ps = ps1.tile([128, TQ], F32, tag="psqk",
                                  name=f"psqk{j}_{m}")
                    for d in range(NDCH):
                        nc.tensor.matmul(
                            ps[:], wqk_sb[d][:, m * 128:(m + 1) * 128],
                            xr[j][d][:], start=(d == 0), stop=(d == NDCH - 1))
                    if m < 4:
                        nc.vector.tensor_scalar_add(
                            qTp[2 * m][0:64, jsl], ps[0:64, :],
                            bqk_sb[m][0:64])
                        nc.vector.tensor_scalar_add(
                            qTp[2 * m + 1][64:128, jsl], ps[64:128, :],
                            bqk_sb[m][64:128])
                    else:
                        nc.vector.tensor_scalar_add(
                            kT[m - 4][:, jsl], ps[:], bqk_sb[m][:])

                for tt in range(4 * j, 4 * j + 4):
                    c = tt % 4
                    ps = psv.tile([128, DL], F32, tag="psv", name=f"psv{tt}")
                    for d in range(NDCH):
                        nc.tensor.matmul(
                            ps[:], xr[j][d][:, c * 128:(c + 1) * 128],
                            wv_sb[d][:], start=(d == 0), stop=False)
                    nc.tensor.matmul(ps[:], ones_r[:, 0:128], bv_r,
                                     start=False, stop=True)
                    for h in range(HL):
                        nc.vector.tensor_copy(
                            vs[tt][:, h * 65:h * 65 + 64],
                            ps[:, h * 64:(h + 1) * 64])

        # -------- phase 2: attention + projection --------
        with ExitStack() as ph2:
            ps_s = ph2.enter_context(tc.tile_pool(name="ps_s", bufs=2,
                                                  space="PSUM"))
            ps_o = ph2.enter_context(tc.tile_pool(name="ps_o", bufs=4,
                                                  space="PSUM"))
            ppool = ph2.enter_context(tc.tile_pool(name="ppool", bufs=6))
            npool = ph2.enter_context(tc.tile_pool(name="npool", bufs=2))
            opool = ph2.enter_context(tc.tile_pool(name="opool", bufs=3))

            pending = []

            def pop_pending():
                if pending:
                    pending.pop(0)()

            def norm_cbs(j, i, poA, poB):
                """Deferred normalization of head pair (2i, 2i+1) of block
                j: divide po rows 0:64 by the softmax sums in row 64.
                recip -> f32r cast -> PE ones-broadcast into a borrowed
                ss-pool tile -> po*pb multiply into yT."""
                jsl = slice(j * TQ, (j + 1) * TQ)
                sumA = npool.tile([1, TQ], F32, tag="sumA", name=f"sa{j}_{i}")
                sumB = npool.tile([1, TQ], F32, tag="sumB", name=f"sb{j}_{i}")
                recAB = npool.tile([1, 2 * TQ], F32, tag="recAB",
                                   name=f"rc{j}_{i}")
                scr = npool.tile([1, TQ], F32, tag="scr", name=f"sc{j}_{i}")
                pbAB = npool.tile([64, 2 * TQ], F32, tag="pbAB",
                                  name=f"pb{j}_{i}")

                def cb1():
                    nc.vector.tensor_copy(sumA[:], poA[64:65, :])
                    nc.vector.tensor_copy(sumB[:], poB[64:65, :])

                def cb2():
                    nc.vector.reciprocal_approx_accurate(
                        out=recAB[:, 0:TQ], in_=sumA[:], scratch=scr[:])
                    nc.vector.reciprocal_approx_accurate(
                        out=recAB[:, TQ:2 * TQ], in_=sumB[:], scratch=scr[:])

                def cb3():
                    nc.gpsimd.partition_broadcast(pbAB[:], recAB[:])

                def cb4():
                    nc.vector.tensor_mul(yT[i][0:64, jsl], poA[0:64, :],
                                         pbAB[:, 0:TQ])
                    nc.vector.tensor_mul(yT[i][64:128, jsl], poB[0:64, :],
                                         pbAB[:, TQ:2 * TQ])

                return [cb1, cb2, cb3, cb4]

            def proj_burst(jb):
                """Output projection of block jb: 8 steps, each borrows an
                ss-pool tile for its PSUM accumulation."""
                for t in range(4 * jb, 4 * jb + 4):
                    for nb in range(2):
                        nsl = slice(nb * 512, (nb + 1) * 512)
                        ps = ps_s.tile([TKC, 2 * TQ], F32, tag="ss",
                                       name=f"ssp{t}_{nb}")
                        for k in range(4):
                            nc.tensor.matmul(
                                ps[:, 0:TQ],
                                yT[k][:, t * 128:(t + 1) * 128],
                                wp_sb[k][:, nsl], start=(k == 0),
                                stop=(k == 3))
                        ot = opool.tile([128, TQ], F32, tag="ot",
                                        name=f"ot{t}_{nb}")
                        nc.vector.tensor_copy(ot[:], ps[:, 0:TQ])
                        nc.sync.dma_start(out_d[t * 128:(t + 1) * 128, nsl],
                                          ot[:])
                        pop_pending()

            for j in range(NQB):
                jsl = slice(j * TQ, (j + 1) * TQ)
                cs = list(range(4 * (j + 1))) if causal else list(range(NKC))
                for i in range(4):          # head pair (2i, 2i+1)
                    if i == 2 and j > 0:
                        proj_burst(j - 1)
                    hA, hB = 2 * i, 2 * i + 1
                    poA = ps_o.tile([128, TQ], F32, tag="po",
                                    name=f"poA{j}_{i}")
                    poB = ps_o.tile([128, TQ], F32, tag="po",
                                    name=f"poB{j}_{i}")

                    pendq = []    # pipeline: PV(c) emitted after QK(c+2)
                    first_pv = [True]

                    def emit_pv(pc, ppt, pq0, stop):
                        st = first_pv[0]
                        first_pv[0] = False
                        nc.tensor.matmul(
                            poA[:, pq0:TQ],
                            vs[pc][:, hA * 65:hA * 65 + 128],
                            ppt[:, pq0:TQ], start=st, stop=stop,
                            skip_group_check=True)
                        nc.tensor.matmul(
                            poB[:, pq0:TQ],
                            vs[pc][:, hB * 65:hB * 65 + 128],
                            ppt[:, TQ + pq0:2 * TQ], start=st, stop=stop,
                            skip_group_check=True)

                    for ci, c in enumerate(cs):
                        diag = causal and c >= 4 * j
                        q0 = (c - 4 * j) * TKC if diag else 0
                        csl = slice(c * TKC, (c + 1) * TKC)
                        ss = ps_s.tile([TKC, 2 * TQ], F32, tag="ss",
                                       name=f"ss{j}_{i}_{c}")
                        nc.tensor.matmul(
                            ss[:, q0:TQ], kT[i][:, csl],
                            qTp[hA][:, j * TQ + q0:(j + 1) * TQ],
                            start=True, stop=True)
                        nc.tensor.matmul(
                            ss[:, TQ + q0:2 * TQ], kT[i][:, csl],
                            qTp[hB][:, j * TQ + q0:(j + 1) * TQ],
                            start=True, stop=True)
                        pt = ppool.tile([TKC, 2 * TQ], BF16, tag="pt",
                                        name=f"pt{j}_{i}_{c}")
                        if q0 == 0:
                            nc.scalar.activation(pt[:], ss[:], EXP,
                                                 scale=0.125)
                        else:
                            nc.scalar.activation(pt[:, q0:TQ], ss[:, q0:TQ],
                                                 EXP, scale=0.125)
                            nc.scalar.activation(pt[:, TQ + q0:2 * TQ],
                                                 ss[:, TQ + q0:2 * TQ],
                                                 EXP, scale=0.125)
                        if diag:
                            # tril mask on the one boundary 128-col block
                            nc.vector.tensor_mul(pt[:, q0:q0 + TKC],
                                                 pt[:, q0:q0 + TKC],
                                                 tril2[:, 0:TKC])
                            nc.vector.tensor_mul(
                                pt[:, TQ + q0:TQ + q0 + TKC],
                                pt[:, TQ + q0:TQ + q0 + TKC],
                                tril2[:, TKC:2 * TKC])
                        if len(pendq) == 3:
                            emit_pv(*pendq.pop(0), stop=False)
                        pop_pending()
                        pendq.append((c, pt, q0))
                    while pendq:
                        emit_pv(*pendq.pop(0), stop=(len(pendq) == 0))

                    cbs = norm_cbs(j, i, poA, poB)
                    if j == NQB - 1 and i == 3:
                        for cb in cbs:   # last pair: start the chain now
                            cb()
                    else:
                        pending.extend(cbs)

            for fn in list(pending):   # flush last block's normalization
                pop_pending()
            proj_burst(NQB - 1)

    nc.compile()
    return nc


def _get_nc(causal: bool):
    if causal not in _CACHE:
        _CACHE[causal] = _build(causal)
    return _CACHE[causal]


def _host_tril2() -> np.ndarray:
    i = np.arange(TKC)[:, None]
    jj = np.arange(TKC)[None, :]
    blk = (jj >= i).astype(np.float32)
    return np.ascontiguousarray(
        np.concatenate([blk, blk], axis=1).astype(ml_dtypes.bfloat16))


def _make_in_maps(x, W_qkv, b_qkv, W_proj):
    tril_np = _host_tril2()
    bf = ml_dtypes.bfloat16
    in_maps = []
    for core in range(N_CORES):
        b, g = core // 2, core % 2
        qc = slice(g * DL, (g + 1) * DL)
        kc = slice(D + g * DL, D + (g + 1) * DL)
        vc = slice(2 * D + g * DL, 2 * D + (g + 1) * DL)
        in_maps.append({
            "xT": np.ascontiguousarray(x[b].T.astype(bf)),
            "wqk": np.ascontiguousarray(
                np.concatenate([W_qkv[:, qc], W_qkv[:, kc]],
                               axis=1).astype(bf)),
            "wv": np.ascontiguousarray(W_qkv[:, vc].astype(bf)),
            "bqk": np.ascontiguousarray(
                np.concatenate([b_qkv[qc], b_qkv[kc]]).reshape(8, 128, 1)),
            "bv": np.ascontiguousarray(b_qkv[vc].reshape(1, DL)),
            "wproj": np.ascontiguousarray(
                W_proj[g * DL:(g + 1) * DL, :].astype(bf)),
            "tril2": tril_np,
        })
    return in_maps


def kernel(x, mask, W_qkv, b_qkv, W_proj, b_proj):
    x = np.asarray(x, dtype=np.float32)
    mask2d = np.asarray(mask, dtype=np.int32).reshape(T, T)
    W_qkv = np.asarray(W_qkv, dtype=np.float32)
    b_qkv = np.asarray(b_qkv, dtype=np.float32)
    W_proj = np.asarray(W_proj, dtype=np.float32)
    b_proj = np.asarray(b_proj, dtype=np.float32)

    if np.array_equal(mask2d, np.tril(np.ones((T, T), dtype=np.int32))):
        causal = True
    elif np.all(mask2d == 1):
        causal = False
    else:
        raise NotImplementedError("only causal (tril) or all-ones masks")

    nc = _get_nc(causal)
    in_maps = _make_in_maps(x, W_qkv, b_qkv, W_proj)
    res = run_bass_kernel_spmd(nc, in_maps, core_ids=list(range(N_CORES)))
    out = np.empty((B, T, D), dtype=np.float32)
    for b in range(B):
        out[b] = (res.results[2 * b]["out"] + res.results[2 * b + 1]["out"]
                  + b_proj[None, :])
    return out
